# revision 2
# baseline (speedup 1.0000x reference)
"""HadLinear TRN2 kernel: out = fwht_1024blocks(x)/sqrt(1024) @ W.T

Math: the blockwise FWHT is multiplication by a symmetric matrix
(blockdiag of H_1024 = H_2^{x10}), so it folds into the weight:
    y = fwht(x)/32 @ W^T = x @ (fwht(W)/32)^T
The weight transform is done once on host (numpy); the device kernel is
a pure bf16 matmul, data-parallel over rows of x (2048 rows/core):
    y_core[2048, 4096] = x_core[2048, 4096] @ Wh[4096, 4096]^T

Device layout (all DMA contiguous, no transposes on device):
  xt  [4096(k), 2048(m)] bf16   - x_core^T, host-transposed
  wt  [4096(k), 4096(n)] bf16   - (fwht(W)/32)^T, host-prepared
  y   [2048(m), 4096(n)] f32

Loop: n-strips of 512 (one PSUM bank per out tile). W streamed once
(strip by strip), x fully SBUF-resident (16MB). Strip 0 runs k-outer
across 8 PSUM banks so the PE starts as soon as the first x k-slab
lands; strips 1..7 run m-outer / k-inner (32 chained matmuls per bank).

Self-contained: hardcodes shapes B=4, S=4096, D_in=D_out=4096, 8 cores.
"""

import math
import numpy as np
import ml_dtypes

import concourse.bacc as bacc
import concourse.mybir as mybir
import concourse.tile as tile
from concourse.bass_utils import run_bass_kernel_spmd

BF16 = ml_dtypes.bfloat16

P = 128
N_CORES = 8
B_FULL, S_FULL, D = 4, 4096, 4096
M_FULL = B_FULL * S_FULL          # 16384 rows total
M_CORE = M_FULL // N_CORES        # 2048 rows per core
HAD = 1024                        # hadamard block
NSTRIP = 512                      # out-feature strip width (PSUM bank)

KT = D // P                       # 32 k-tiles
MT = M_CORE // P                  # 16 m-tiles
NS = D // NSTRIP                  # 8 n-strips


def build_nc():
    f32, bf16 = mybir.dt.float32, mybir.dt.bfloat16
    nc = bacc.Bacc(None, target_bir_lowering=False, debug=False)

    xt = nc.declare_dram_parameter("xt", [D, M_CORE], bf16, isOutput=False)
    wt = nc.declare_dram_parameter("wt", [D, D], bf16, isOutput=False)
    y = nc.declare_dram_parameter("y", [M_CORE, D], f32, isOutput=True)

    with tile.TileContext(nc) as tc:
        with (
            tc.tile_pool(name="xp", bufs=KT) as xp,          # 32 x 4KB/part
            tc.tile_pool(name="wp", bufs=KT + 16) as wp,     # 48 x 1KB/part
            tc.tile_pool(name="op", bufs=6) as op,           # 6 x 2KB/part
            tc.tile_pool(name="ps", bufs=8, space="PSUM") as psp,
        ):
            # x^T k-slabs, loaded once, resident for the whole kernel
            xtiles = []
            for kt_i in range(KT):
                t = xp.tile([P, M_CORE], bf16, tag="x", name=f"x_{kt_i}")
                nc.sync.dma_start(out=t[:], in_=xt[kt_i * P:(kt_i + 1) * P, :])
                xtiles.append(t)

            def evict(ps_tile, m, ns):
                cout = op.tile([P, NSTRIP], f32, tag="o", name=f"o_{ns}_{m}")
                nc.any.tensor_copy(out=cout[:], in_=ps_tile[:])
                nc.sync.dma_start(
                    out=y[m * P:(m + 1) * P, ns * NSTRIP:(ns + 1) * NSTRIP],
                    in_=cout[:])

            for ns in range(NS):
                n0 = ns * NSTRIP
                wtiles = []
                for kt_i in range(KT):
                    t = wp.tile([P, NSTRIP], bf16, tag="w",
                                name=f"w_{ns}_{kt_i}")
                    nc.sync.dma_start(
                        out=t[:], in_=wt[kt_i * P:(kt_i + 1) * P,
                                         n0:n0 + NSTRIP])
                    wtiles.append(t)

                if ns == 0:
                    # k-outer over 8-bank m-groups: compute starts on the
                    # first (x, w) k-slab instead of waiting for all of x
                    for half in range(2):
                        pss = [psp.tile([P, NSTRIP], f32, tag="ps",
                                        name=f"ps0_{half}_{g}")
                               for g in range(8)]
                        for kt_i in range(KT):
                            for g in range(8):
                                m = half * 8 + g
                                nc.tensor.matmul(
                                    pss[g][:],
                                    lhsT=xtiles[kt_i][:, m * P:(m + 1) * P],
                                    rhs=wtiles[kt_i][:],
                                    start=(kt_i == 0), stop=(kt_i == KT - 1))
                        for g in range(8):
                            evict(pss[g], half * 8 + g, ns)
                else:
                    # m-outer, k-inner: 32 chained matmuls per PSUM bank
                    for m in range(MT):
                        ps = psp.tile([P, NSTRIP], f32, tag="ps",
                                      name=f"ps_{ns}_{m}")
                        for kt_i in range(KT):
                            nc.tensor.matmul(
                                ps[:],
                                lhsT=xtiles[kt_i][:, m * P:(m + 1) * P],
                                rhs=wtiles[kt_i][:],
                                start=(kt_i == 0), stop=(kt_i == KT - 1))
                        evict(ps, m, ns)
    nc.compile()
    return nc


_CACHE = {}


def _get_nc():
    if "nc" not in _CACHE:
        _CACHE["nc"] = build_nc()
    return _CACHE["nc"]


def _fwht_rows(a):
    """FWHT along last axis (matches reference ordering), float32."""
    orig = a.shape
    n = orig[-1]
    a = np.ascontiguousarray(a).reshape(-1, n)
    h = 1
    while h < n:
        v = a.reshape(-1, 2, h)
        s = v[:, 0, :] + v[:, 1, :]
        d = v[:, 0, :] - v[:, 1, :]
        v[:, 0, :] = s
        v[:, 1, :] = d
        h *= 2
    return a.reshape(orig)


def _prep_inputs(x, weight):
    """Host prep: fold FWHT into W, transpose + cast to bf16."""
    x2d = np.asarray(x, dtype=np.float32).reshape(M_FULL, D)
    w = np.asarray(weight, dtype=np.float32)

    wh = _fwht_rows(w.reshape(D, D // HAD, HAD)).reshape(D, D)
    wh *= 1.0 / math.sqrt(HAD)
    wt_bf = np.ascontiguousarray(wh.T).astype(BF16)

    xbf = x2d.astype(BF16)
    xts = [np.ascontiguousarray(xbf[c * M_CORE:(c + 1) * M_CORE, :].T)
           for c in range(N_CORES)]
    return xts, wt_bf


def run(x, weight, trace=False):
    assert x.shape == (B_FULL, S_FULL, D) and weight.shape == (D, D)
    nc = _get_nc()
    xts, wt_bf = _prep_inputs(x, weight)
    in_maps = [{"xt": xts[c], "wt": wt_bf} for c in range(N_CORES)]
    res = run_bass_kernel_spmd(nc, in_maps, core_ids=list(range(N_CORES)),
                               trace=trace)
    yv = np.concatenate([r["y"] for r in res.results], axis=0)
    return yv.reshape(B_FULL, S_FULL, D), res


def kernel(x, weight):
    return run(x, weight)[0]


# revision 3
# speedup vs baseline: 1.0376x; 1.0376x over previous
"""HadLinear TRN2 kernel: out = fwht_1024blocks(x)/sqrt(1024) @ W.T

Math: the blockwise FWHT is multiplication by a symmetric matrix
(blockdiag of H_1024 = H_2^{x10}), so it folds into the weight:
    y = fwht(x)/32 @ W^T = x @ (fwht(W)/32)^T
The weight transform is done once on host (numpy); the device kernel is
a pure bf16 matmul, data-parallel over rows of x (2048 rows/core):
    y_core[2048, 4096] = x_core[2048, 4096] @ Wh[4096, 4096]^T

Device layout (all DMA contiguous, no transposes on device):
  xt  [4096(k), 2048(m)] bf16   - x_core^T, host-transposed
  wt  [4096(k), 4096(n)] bf16   - (fwht(W)/32)^T, host-prepared
  y   [2048(m), 4096(n)] f32

Loop: n-strips of 512 (one PSUM bank per out tile). W streamed once
(strip by strip), x fully SBUF-resident (16MB). Strip 0 runs k-outer
across 8 PSUM banks so the PE starts as soon as the first x k-slab
lands; strips 1..7 run m-outer / k-inner (32 chained matmuls per bank).

Self-contained: hardcodes shapes B=4, S=4096, D_in=D_out=4096, 8 cores.
"""

import math
import numpy as np
import ml_dtypes

import concourse.bacc as bacc
import concourse.mybir as mybir
import concourse.tile as tile
from concourse.bass_utils import run_bass_kernel_spmd

BF16 = ml_dtypes.bfloat16

P = 128
N_CORES = 8
B_FULL, S_FULL, D = 4, 4096, 4096
M_FULL = B_FULL * S_FULL          # 16384 rows total
M_CORE = M_FULL // N_CORES        # 2048 rows per core
HAD = 1024                        # hadamard block
NSTRIP = 512                      # out-feature strip width (PSUM bank)

KT = D // P                       # 32 k-tiles
MT = M_CORE // P                  # 16 m-tiles
NS = D // NSTRIP                  # 8 n-strips


def build_nc():
    f32, bf16 = mybir.dt.float32, mybir.dt.bfloat16
    nc = bacc.Bacc(None, target_bir_lowering=False, debug=False)

    xt = nc.declare_dram_parameter("xt", [D, M_CORE], bf16, isOutput=False)
    wt = nc.declare_dram_parameter("wt", [D, D], bf16, isOutput=False)
    y = nc.declare_dram_parameter("y", [M_CORE, D], f32, isOutput=True)

    with tile.TileContext(nc) as tc:
        with (
            tc.tile_pool(name="xp", bufs=KT) as xp,          # 32 x 4KB/part
            tc.tile_pool(name="wp", bufs=KT + 16) as wp,     # 48 x 1KB/part
            tc.tile_pool(name="op", bufs=6) as op,           # 6 x 2KB/part
            tc.tile_pool(name="ps", bufs=8, space="PSUM") as psp,
        ):
            # Startup: interleave x^T k-slabs with strip-0 W tiles (k-paired)
            # on the same DMA queue, so strip-0's k-outer matmuls start on
            # the first pair instead of waiting behind the whole x load.
            xtiles = []
            w0tiles = []
            for kt_i in range(KT):
                t = xp.tile([P, M_CORE], bf16, tag="x", name=f"x_{kt_i}")
                nc.sync.dma_start(out=t[:], in_=xt[kt_i * P:(kt_i + 1) * P, :])
                xtiles.append(t)
                w = wp.tile([P, NSTRIP], bf16, tag="w", name=f"w_0_{kt_i}")
                nc.sync.dma_start(
                    out=w[:], in_=wt[kt_i * P:(kt_i + 1) * P, 0:NSTRIP])
                w0tiles.append(w)

            def evict(ps_tile, m, ns):
                cout = op.tile([P, NSTRIP], f32, tag="o", name=f"o_{ns}_{m}")
                nc.any.tensor_copy(out=cout[:], in_=ps_tile[:])
                # gpsimd queue: keep y stores off the x/W load queue
                nc.gpsimd.dma_start(
                    out=y[m * P:(m + 1) * P, ns * NSTRIP:(ns + 1) * NSTRIP],
                    in_=cout[:])

            for ns in range(NS):
                n0 = ns * NSTRIP
                if ns == 0:
                    wtiles = w0tiles
                else:
                    wtiles = []
                    for kt_i in range(KT):
                        t = wp.tile([P, NSTRIP], bf16, tag="w",
                                    name=f"w_{ns}_{kt_i}")
                        nc.sync.dma_start(
                            out=t[:], in_=wt[kt_i * P:(kt_i + 1) * P,
                                             n0:n0 + NSTRIP])
                        wtiles.append(t)

                if ns == 0:
                    # k-outer over 8-bank m-groups: compute starts on the
                    # first (x, w) k-slab instead of waiting for all of x
                    for half in range(2):
                        pss = [psp.tile([P, NSTRIP], f32, tag="ps",
                                        name=f"ps0_{half}_{g}")
                               for g in range(8)]
                        for kt_i in range(KT):
                            for g in range(8):
                                m = half * 8 + g
                                nc.tensor.matmul(
                                    pss[g][:],
                                    lhsT=xtiles[kt_i][:, m * P:(m + 1) * P],
                                    rhs=wtiles[kt_i][:],
                                    start=(kt_i == 0), stop=(kt_i == KT - 1))
                        for g in range(8):
                            evict(pss[g], half * 8 + g, ns)
                else:
                    # m-outer, k-inner: 32 chained matmuls per PSUM bank
                    for m in range(MT):
                        ps = psp.tile([P, NSTRIP], f32, tag="ps",
                                      name=f"ps_{ns}_{m}")
                        for kt_i in range(KT):
                            nc.tensor.matmul(
                                ps[:],
                                lhsT=xtiles[kt_i][:, m * P:(m + 1) * P],
                                rhs=wtiles[kt_i][:],
                                start=(kt_i == 0), stop=(kt_i == KT - 1))
                        evict(ps, m, ns)
    nc.compile()
    return nc


_CACHE = {}


def _get_nc():
    if "nc" not in _CACHE:
        _CACHE["nc"] = build_nc()
    return _CACHE["nc"]


def _fwht_rows(a):
    """FWHT along last axis (matches reference ordering), float32."""
    orig = a.shape
    n = orig[-1]
    a = np.ascontiguousarray(a).reshape(-1, n)
    h = 1
    while h < n:
        v = a.reshape(-1, 2, h)
        s = v[:, 0, :] + v[:, 1, :]
        d = v[:, 0, :] - v[:, 1, :]
        v[:, 0, :] = s
        v[:, 1, :] = d
        h *= 2
    return a.reshape(orig)


def _prep_inputs(x, weight):
    """Host prep: fold FWHT into W, transpose + cast to bf16."""
    x2d = np.asarray(x, dtype=np.float32).reshape(M_FULL, D)
    w = np.asarray(weight, dtype=np.float32)

    wh = _fwht_rows(w.reshape(D, D // HAD, HAD)).reshape(D, D)
    wh *= 1.0 / math.sqrt(HAD)
    wt_bf = np.ascontiguousarray(wh.T).astype(BF16)

    xbf = x2d.astype(BF16)
    xts = [np.ascontiguousarray(xbf[c * M_CORE:(c + 1) * M_CORE, :].T)
           for c in range(N_CORES)]
    return xts, wt_bf


def run(x, weight, trace=False):
    assert x.shape == (B_FULL, S_FULL, D) and weight.shape == (D, D)
    nc = _get_nc()
    xts, wt_bf = _prep_inputs(x, weight)
    in_maps = [{"xt": xts[c], "wt": wt_bf} for c in range(N_CORES)]
    res = run_bass_kernel_spmd(nc, in_maps, core_ids=list(range(N_CORES)),
                               trace=trace)
    yv = np.concatenate([r["y"] for r in res.results], axis=0)
    return yv.reshape(B_FULL, S_FULL, D), res


def kernel(x, weight):
    return run(x, weight)[0]


# revision 4
# speedup vs baseline: 1.0483x; 1.0103x over previous
"""HadLinear TRN2 kernel: out = fwht_1024blocks(x)/sqrt(1024) @ W.T

Math: the blockwise FWHT is multiplication by a symmetric matrix
(blockdiag of H_1024 = H_2^{x10}), so it folds into the weight:
    y = fwht(x)/32 @ W^T = x @ (fwht(W)/32)^T
The weight transform is done once on host (numpy); the device kernel is
a pure bf16 matmul, data-parallel over rows of x (2048 rows/core):
    y_core[2048, 4096] = x_core[2048, 4096] @ Wh[4096, 4096]^T

Device layout (all DMA contiguous, no transposes on device):
  xt  [4096(k), 2048(m)] bf16   - x_core^T, host-transposed
  wt  [4096(k), 4096(n)] bf16   - (fwht(W)/32)^T, host-prepared
  y   [2048(m), 4096(n)] f32

Loop: n-strips of 512 (one PSUM bank per out tile). W streamed once
(strip by strip), x fully SBUF-resident (16MB). Strip 0 runs k-outer
across 8 PSUM banks so the PE starts as soon as the first x k-slab
lands; strips 1..7 run m-outer / k-inner (32 chained matmuls per bank).

Self-contained: hardcodes shapes B=4, S=4096, D_in=D_out=4096, 8 cores.
"""

import math
import numpy as np
import ml_dtypes

import concourse.bacc as bacc
import concourse.mybir as mybir
import concourse.tile as tile
from concourse.bass_utils import run_bass_kernel_spmd

BF16 = ml_dtypes.bfloat16

P = 128
N_CORES = 8
B_FULL, S_FULL, D = 4, 4096, 4096
M_FULL = B_FULL * S_FULL          # 16384 rows total
M_CORE = M_FULL // N_CORES        # 2048 rows per core
HAD = 1024                        # hadamard block
NSTRIP = 512                      # out-feature strip width (PSUM bank)

KT = D // P                       # 32 k-tiles
MT = M_CORE // P                  # 16 m-tiles
NS = D // NSTRIP                  # 8 n-strips


def build_nc():
    f32, bf16 = mybir.dt.float32, mybir.dt.bfloat16
    nc = bacc.Bacc(None, target_bir_lowering=False, debug=False)

    xt = nc.declare_dram_parameter("xt", [D, M_CORE], bf16, isOutput=False)
    wt = nc.declare_dram_parameter("wt", [D, D], bf16, isOutput=False)
    y = nc.declare_dram_parameter("y", [M_CORE, D], f32, isOutput=True)

    MH = M_CORE // 2              # 1024: x column-half width
    with tile.TileContext(nc) as tc:
        with (
            tc.tile_pool(name="xp", bufs=2 * KT) as xp,      # 64 x 2KB/part
            tc.tile_pool(name="wp", bufs=2 * KT) as wp,      # 64 x 1KB/part
            tc.tile_pool(name="op", bufs=4) as op,           # 4 x 2KB/part
            tc.tile_pool(name="ps", bufs=8, space="PSUM") as psp,
        ):
            # Startup DMA order (single queue): (xA slab, W0 tile) pairs in
            # k order, then xB slabs.  Strip-0 phase A consumes the pairs
            # at 1.31us/kt DMA vs 1.73us/kt of matmul -> PE-bound from the
            # first k-slab; phase B then consumes xB at 0.73us/kt.
            # [128, 1024] x-slabs give 2KB DMA lines (full HBM rate).
            xa, xb, w0tiles = [], [], []
            for kt_i in range(KT):
                t = xp.tile([P, MH], bf16, tag="x", name=f"xa_{kt_i}")
                nc.sync.dma_start(out=t[:],
                                  in_=xt[kt_i * P:(kt_i + 1) * P, 0:MH])
                xa.append(t)
                w = wp.tile([P, NSTRIP], bf16, tag="w", name=f"w_0_{kt_i}")
                nc.sync.dma_start(
                    out=w[:], in_=wt[kt_i * P:(kt_i + 1) * P, 0:NSTRIP])
                w0tiles.append(w)
            for kt_i in range(KT):
                t = xp.tile([P, MH], bf16, tag="x", name=f"xb_{kt_i}")
                nc.sync.dma_start(out=t[:],
                                  in_=xt[kt_i * P:(kt_i + 1) * P, MH:M_CORE])
                xb.append(t)

            def lhs(kt_i, m):
                half, sub = divmod(m, MT // 2)
                src = xa[kt_i] if half == 0 else xb[kt_i]
                return src[:, sub * P:(sub + 1) * P]

            def evict(ps_tile, m, ns):
                # alternate scalar/vector so eviction copies of adjacent
                # banks run in parallel (different PSUM banks: legal)
                cout = op.tile([P, NSTRIP], f32, tag="o", name=f"o_{ns}_{m}")
                if m % 2 == 0:
                    nc.scalar.copy(out=cout[:], in_=ps_tile[:])
                else:
                    nc.vector.tensor_copy(out=cout[:], in_=ps_tile[:])
                # gpsimd queue: keep y stores off the x/W load queue
                nc.gpsimd.dma_start(
                    out=y[m * P:(m + 1) * P, ns * NSTRIP:(ns + 1) * NSTRIP],
                    in_=cout[:])

            for ns in range(NS):
                n0 = ns * NSTRIP
                if ns == 0:
                    wtiles = w0tiles
                else:
                    wtiles = []
                    for kt_i in range(KT):
                        t = wp.tile([P, NSTRIP], bf16, tag="w",
                                    name=f"w_{ns}_{kt_i}")
                        nc.sync.dma_start(
                            out=t[:], in_=wt[kt_i * P:(kt_i + 1) * P,
                                             n0:n0 + NSTRIP])
                        wtiles.append(t)

                if ns == 0:
                    # k-outer over 8-bank m-groups: compute starts on the
                    # first (x, w) k-slab instead of waiting for all of x.
                    # Evictions fire per-bank right after that bank's last
                    # matmul so the next phase never waits on a bank.
                    for half in range(2):
                        pss = [psp.tile([P, NSTRIP], f32, tag="ps",
                                        name=f"ps0_{half}_{g}")
                               for g in range(8)]
                        for kt_i in range(KT):
                            last = kt_i == KT - 1
                            for g in range(8):
                                m = half * 8 + g
                                nc.tensor.matmul(
                                    pss[g][:],
                                    lhsT=lhs(kt_i, m),
                                    rhs=wtiles[kt_i][:],
                                    start=(kt_i == 0), stop=last)
                                if last:
                                    evict(pss[g], m, ns)
                else:
                    # m-outer, k-inner: 32 chained matmuls per PSUM bank
                    for m in range(MT):
                        ps = psp.tile([P, NSTRIP], f32, tag="ps",
                                      name=f"ps_{ns}_{m}")
                        for kt_i in range(KT):
                            nc.tensor.matmul(
                                ps[:],
                                lhsT=lhs(kt_i, m),
                                rhs=wtiles[kt_i][:],
                                start=(kt_i == 0), stop=(kt_i == KT - 1))
                        evict(ps, m, ns)
    nc.compile()
    return nc


_CACHE = {}


def _get_nc():
    if "nc" not in _CACHE:
        _CACHE["nc"] = build_nc()
    return _CACHE["nc"]


def _fwht_rows(a):
    """FWHT along last axis (matches reference ordering), float32."""
    orig = a.shape
    n = orig[-1]
    a = np.ascontiguousarray(a).reshape(-1, n)
    h = 1
    while h < n:
        v = a.reshape(-1, 2, h)
        s = v[:, 0, :] + v[:, 1, :]
        d = v[:, 0, :] - v[:, 1, :]
        v[:, 0, :] = s
        v[:, 1, :] = d
        h *= 2
    return a.reshape(orig)


def _prep_inputs(x, weight):
    """Host prep: fold FWHT into W, transpose + cast to bf16."""
    x2d = np.asarray(x, dtype=np.float32).reshape(M_FULL, D)
    w = np.asarray(weight, dtype=np.float32)

    wh = _fwht_rows(w.reshape(D, D // HAD, HAD)).reshape(D, D)
    wh *= 1.0 / math.sqrt(HAD)
    wt_bf = np.ascontiguousarray(wh.T).astype(BF16)

    xbf = x2d.astype(BF16)
    xts = [np.ascontiguousarray(xbf[c * M_CORE:(c + 1) * M_CORE, :].T)
           for c in range(N_CORES)]
    return xts, wt_bf


def run(x, weight, trace=False):
    assert x.shape == (B_FULL, S_FULL, D) and weight.shape == (D, D)
    nc = _get_nc()
    xts, wt_bf = _prep_inputs(x, weight)
    in_maps = [{"xt": xts[c], "wt": wt_bf} for c in range(N_CORES)]
    res = run_bass_kernel_spmd(nc, in_maps, core_ids=list(range(N_CORES)),
                               trace=trace)
    yv = np.concatenate([r["y"] for r in res.results], axis=0)
    return yv.reshape(B_FULL, S_FULL, D), res


def kernel(x, weight):
    return run(x, weight)[0]
